# revision 44
# baseline (speedup 1.0000x reference)
"""Causal self-attention (B=2, S=2048, D=2048, H=16) on 8 TRN2 NeuronCores.

Sharding: tensor-parallel over heads (2 heads/core) for QKV projection and
attention; per-batch AllToAll redistributes per-head context to per-row
shards (overlapped with the next batch's compute); the output projection is
row-parallel; the host concatenates the 8 row shards.

Data layouts (per core c, heads h0=2c, h1=2c+1):
  xt    [D, R]     X^T, full (R = B*S rows), fp16
  wqk   [D, 512]   w_qkv columns [q_h0 | q_h1 | k_h0 | k_h1] (128 each), fp16
  wv    [D, 256]   w_qkv columns [v_h0 | v_h1], fp16
  wo    [D, D]     full output projection weight, fp16
  masks [128, 128] single causal triangle m[p,f] = (f >= p), fp16

Pipeline (per core, per batch b):
  A) stream xt per q-block (512 rows), QKV via PE chains; Q tiles stay in
     SBUF, K/V cached per head (ACT drains PSUM).
  B) per (h, q-block): scores k-tiles (diagonal tiles first, q-sliced to the
     causal extent), exp on ACT (no max subtraction: scores ~ N(0,1)),
     triangle mask on the 128-wide diagonal strip only, ctx accumulation in
     PSUM, fp16 denominator accumulation on DVE, reciprocal + Pool-engine
     partition_broadcast, normalize+cast fused on DVE.
  C) after batch b: AllToAll of ctx (256-row chunks) overlapped with batch
     b+1 compute; row-parallel out-proj from resident wo; bf16 outT.

Output outT [D, 512] bf16 per core; columns = [b0 rows c*256..(c+1)*256,
b1 rows 2048+c*256..]; host reassembles and casts to f32.
"""

import numpy as np

import concourse.bass as bass
import concourse.mybir as mybir
import concourse.tile as tile
from concourse import bacc
from concourse.bass_utils import run_bass_kernel_spmd

F32 = mybir.dt.float32
F32R = mybir.dt.float32r
F16 = mybir.dt.float16
BF = mybir.dt.bfloat16
AF = mybir.ActivationFunctionType

N_CORES = 8
D = 2048
H = 16
DK = 128
HPC = H // N_CORES  # heads per core = 2
SCALE = 1.0 / float(DK) ** 0.5


def build_attention_nc(B, S, with_qkv_bias=False, with_o_bias=False, with_kmask=False,
                       use_collective=True, phases="FC", repeat=1,
                       bcast_pe=True, flush_sync=False, v_transpose=False):
    R = B * S
    RC = R // N_CORES          # out rows per core (512)
    RB = S // N_CORES          # rows per core per batch (256)
    KD = D // 128              # contraction tiles (16)
    NQ = S // 512              # q-blocks per batch (4)
    NKT = S // 128             # k-tiles per batch (16)
    assert R % N_CORES == 0 and S % 512 == 0 and RB * B == RC

    nc = bacc.Bacc(
        "TRN2", target_bir_lowering=False, debug=False, num_devices=N_CORES
    )

    xt = nc.dram_tensor("xt", [D, R], F16, kind="ExternalInput")
    wqk = nc.dram_tensor("wqk", [D, 4 * 128], F16, kind="ExternalInput")
    wv = nc.dram_tensor("wv", [D, 2 * 128], F16, kind="ExternalInput")
    wo = nc.dram_tensor("wo", [D, D], F16, kind="ExternalInput")
    masks = nc.dram_tensor("masks", [128, 128], F16, kind="ExternalInput")
    eye = nc.dram_tensor("eye", [128, 128], F16, kind="ExternalInput")
    if with_qkv_bias:
        bqkT = nc.dram_tensor("bqkT", [128, 4], F32, kind="ExternalInput")
        bvrow = nc.dram_tensor("bvrow", [1, 256], F16, kind="ExternalInput")
    if with_o_bias:
        boT = nc.dram_tensor("boT", [128, KD], F32, kind="ExternalInput")
    if with_kmask:
        kmaskT = nc.dram_tensor("kmaskT", [128, B * NKT], F16, kind="ExternalInput")
    outT = nc.dram_tensor("outT", [D, RC], BF, kind="ExternalOutput")

    with tile.TileContext(nc, num_cores=N_CORES) as tc:
        with tc.tile_pool(name="dram", bufs=1, space="DRAM") as dpool, \
             tc.tile_pool(name="pf", bufs=1) as pf, \
             tc.tile_pool(name="psf", bufs=1, space="PSUM") as psf:
            ctxl = [dpool.tile([N_CORES, HPC * 128, RB], F16, name=f"ctxl_b{b}")
                    for b in range(B)]
            a2a = [dpool.tile([N_CORES, HPC * 128, RB], F16, name=f"a2a_b{b}")
                   for b in range(B)]

            for rep in range(repeat):
                ones = pf.tile([128, 128], F16, name="ones", tag="ones", bufs=1)
                nc.vector.memset(ones[:], 1.0)
                if bcast_pe:
                    ones32 = pf.tile([1, 128], F32, name="ones32",
                                     tag="ones32", bufs=1)
                    nc.vector.memset(ones32[:], 1.0)
                masks_sb = pf.tile([128, 128], F16, name="masks_sb",
                                   tag="masks", bufs=1)
                eye_sb = pf.tile([128, 128], F16, name="eye_sb",
                                 tag="eye", bufs=1)
                if v_transpose:
                    nc.sync.dma_start(eye_sb[:], eye.ap())
                if with_qkv_bias:
                    bqk_sb = pf.tile([128, 4], F32, name="bqk_sb", tag="bqk", bufs=1)
                    nc.sync.dma_start(bqk_sb[:], bqkT.ap())
                    bv_sb = pf.tile([1, 256], F16, name="bv_sb", tag="bv", bufs=1)
                    nc.sync.dma_start(bv_sb[:], bvrow.ap())
                if with_o_bias:
                    bo_sb = pf.tile([128, KD], F32, name="bo_sb", tag="bo", bufs=1)
                    nc.sync.dma_start(bo_sb[:], boT.ap())
                if with_kmask:
                    km_sb = pf.tile([128, B * NKT], F16, name="km_sb",
                                    tag="km", bufs=1)
                    nc.sync.dma_start(km_sb[:], kmaskT.ap())

                wqk_sb = pf.tile([128, KD, 512], F16, name="wqk_sb",
                                 tag="wqk", bufs=1)
                wqk_r = wqk.ap().rearrange("(t p) m -> p t m", p=128)
                wv_sb = pf.tile([128, KD, 256], F16, name="wv_sb",
                                tag="wv", bufs=1)
                wo_sb = pf.tile([128, KD, D], F16, name="wo_sb", tag="wo", bufs=1)
                xt_r = xt.ap().rearrange("(t p) m -> p t m", p=128)
                kcache = [
                    pf.tile([128, S], F16, name=f"kcache{h}", tag=f"kc{h}", bufs=1)
                    for h in range(HPC)
                ]
                vcache = [
                    pf.tile([128, NKT, 128], F16, name=f"vcache{h}",
                            tag=f"vc{h}", bufs=1)
                    for h in range(HPC)
                ]

                # ---------------- Phase F: QKV + attention ----------------
                cfull_all = pf.tile([128, KD, RC], F16, name="cfull",
                                    tag="cfull", bufs=1)

                def emit_outproj():
                    for ob in range(KD):
                        pso = psf.tile([128, RC], F32, name="pso",
                                       tag="mm", bufs=2)
                        for kt in range(KD):
                            nc.tensor.matmul(
                                pso[:],
                                wo_sb[:, kt, ob * 128:(ob + 1) * 128],
                                cfull_all[:, kt, :],
                                start=(kt == 0),
                                stop=(kt == KD - 1),
                            )
                        evo = pf.tile([128, RC], BF, name="evo",
                                      tag="evo", bufs=4)
                        with nc.allow_low_precision(reason="bf16 out"):
                            if with_o_bias:
                                nc.vector.tensor_scalar_add(
                                    evo[:], pso[:], bo_sb[:, ob:ob + 1]
                                )
                            else:
                                nc.scalar.copy(evo[:], pso[:])
                        nc.sync.dma_start(
                            outT.ap()[ob * 128:(ob + 1) * 128, :],
                            evo[:],
                        )

                if "F" in phases:
                 for b in range(B):
                    for qb in range(NQ):
                        xt_q = pf.tile([128, KD, 512], F16, name="xt_q",
                                       tag="xt", bufs=3)
                        xcols = slice(b * S + qb * 512, b * S + (qb + 1) * 512)
                        if b == 0 and qb == 0:
                            # startup: chunked so the first Q/K chain pair
                            # streams with the DMA; chunk size balances the
                            # per-DMA descriptor cost vs transfer latency
                            nc.sync.dma_start(
                                wqk_sb[:, 0:4, 0:256], wqk_r[:, 0:4, 0:256]
                            )
                            nc.sync.dma_start(
                                xt_q[:, 0:4, :], xt_r[:, 0:4, xcols]
                            )
                            nc.sync.dma_start(
                                xt_q[:, 4:8, :], xt_r[:, 4:8, xcols]
                            )
                            nc.sync.dma_start(
                                wqk_sb[:, 4:16, 0:256], wqk_r[:, 4:16, 0:256]
                            )
                            nc.sync.dma_start(
                                xt_q[:, 8:12, :], xt_r[:, 8:12, xcols]
                            )
                            nc.sync.dma_start(
                                wqk_sb[:, :, 256:512], wqk_r[:, :, 256:512]
                            )
                            nc.sync.dma_start(
                                xt_q[:, 12:16, :], xt_r[:, 12:16, xcols]
                            )
                            nc.sync.dma_start(masks_sb[:], masks.ap())
                            # wv only needed once Q/K chains are in flight
                            nc.sync.dma_start(
                                wv_sb[:],
                                wv.ap().rearrange("(t p) m -> p t m", p=128),
                            )
                        else:
                            for g in range(4):
                                nc.sync.dma_start(
                                    xt_q[:, g * 4:(g + 1) * 4, :],
                                    xt_r[:, g * 4:(g + 1) * 4, xcols],
                                )
                        # Q/K chains: m in (q_h0, q_h1, k_h0, k_h1), emitted
                        # k-major over m-pairs so PE streams with arriving xt
                        qtile = []
                        for m0 in (0, 2):
                            pss = [psf.tile([128, 512], F32, name="ps",
                                            tag="mm", bufs=2) for _ in range(2)]
                            for k in range(KD):
                                for mi in range(2):
                                    nc.tensor.matmul(
                                        pss[mi][:],
                                        wqk_sb[:, k, (m0 + mi) * 128:
                                               (m0 + mi + 1) * 128],
                                        xt_q[:, k, :],
                                        start=(k == 0),
                                        stop=(k == KD - 1),
                                    )
                            for mi in range(2):
                                m = m0 + mi
                                if m < 2:
                                    qt = pf.tile([128, 512], F16, name="qt",
                                                 tag="qt", bufs=4)
                                    qtile.append(qt)
                                    dst = qt[:]
                                else:
                                    dst = kcache[m - 2][:, qb * 512:
                                                        (qb + 1) * 512]
                                with nc.allow_low_precision(reason="f16 cache"):
                                    if with_qkv_bias:
                                        nc.vector.tensor_scalar_add(
                                            dst, pss[mi][:], bqk_sb[:, m:m + 1]
                                        )
                                    else:
                                        nc.scalar.copy(dst, pss[mi][:])
                        # V chains
                        if v_transpose and not with_qkv_bias:
                            # feat-major chains (free=512, engine-bound on the
                            # PE sequencer) + PE transpose into row-major cache
                            for h in range(HPC):
                                psvT = psf.tile([128, 512], F32, name="psvT",
                                                tag="mm", bufs=2)
                                for k in range(KD):
                                    nc.tensor.matmul(
                                        psvT[:],
                                        wv_sb[:, k, h * 128:(h + 1) * 128],
                                        xt_q[:, k, :],
                                        start=(k == 0),
                                        stop=(k == KD - 1),
                                    )
                                vT = pf.tile([128, 512], F16, name="vT",
                                             tag="vT", bufs=2)
                                with nc.allow_low_precision(reason="f16 cache"):
                                    nc.scalar.copy(vT[:], psvT[:])
                                for s4 in range(4):
                                    ps_t = psf.tile([128, 128], F16, name="ps_t",
                                                    tag="mmv", bufs=1)
                                    nc.tensor.transpose(
                                        ps_t[:],
                                        vT[:, s4 * 128:(s4 + 1) * 128],
                                        eye_sb[:],
                                    )
                                    nc.scalar.copy(
                                        vcache[h][:, qb * 4 + s4, :], ps_t[:]
                                    )
                        else:
                            for s4 in range(4):
                                psv = psf.tile([128, 256], F32, name="psv",
                                               tag="mmv", bufs=1)
                                for k in range(KD):
                                    nc.tensor.matmul(
                                        psv[:],
                                        xt_q[:, k, s4 * 128:(s4 + 1) * 128],
                                        wv_sb[:, k, :],
                                        start=(k == 0),
                                        stop=(k == KD - 1) and not with_qkv_bias,
                                        skip_group_check=with_qkv_bias,
                                    )
                                if with_qkv_bias:
                                    nc.tensor.matmul(
                                        psv[:], ones[0:1, :], bv_sb[:],
                                        start=False, stop=True,
                                        skip_group_check=True,
                                    )
                                kt_ = qb * 4 + s4
                                for h in range(HPC):
                                    with nc.allow_low_precision(reason="f16 cache"):
                                        nc.scalar.copy(
                                            vcache[h][:, kt_, :],
                                            psv[:, h * 128:(h + 1) * 128],
                                        )
                        # attention for q-block qb, both heads
                        nk = (qb + 1) * 4
                        korder = (
                            list(range(qb * 4, nk)) + list(range(0, qb * 4))
                        )
                        for h in range(HPC):
                            ctx = psf.tile([128, 512], F32, name="ctx",
                                           tag="ctx", bufs=2)
                            dacc = pf.tile([128, 512], F16, name="dacc",
                                           tag="dacc", bufs=2)
                            for idx, kt in enumerate(korder):
                                j = kt - qb * 4  # >=0 on the causal diagonal
                                q0 = j * 128 if j >= 0 else 0
                                sp = psf.tile([128, 512], F32, name="sp",
                                              tag="sp", bufs=2)
                                nc.tensor.matmul(
                                    sp[:, q0:],
                                    kcache[h][:, kt * 128:(kt + 1) * 128],
                                    qtile[h][:, q0:],
                                    start=True,
                                    stop=True,
                                )
                                p = pf.tile([128, 512], F16, name="p",
                                            tag="p", bufs=6)
                                nc.scalar.activation(
                                    p[:, q0:], sp[:, q0:], AF.Exp, scale=SCALE
                                )
                                if j >= 0:
                                    nc.vector.tensor_mul(
                                        p[:, q0:q0 + 128], p[:, q0:q0 + 128],
                                        masks_sb[:],
                                    )
                                if with_kmask:
                                    nc.vector.tensor_scalar_mul(
                                        p[:, q0:], p[:, q0:],
                                        km_sb[:, b * NKT + kt: b * NKT + kt + 1],
                                    )
                                nc.tensor.matmul(
                                    ctx[:, q0:], vcache[h][:, kt, :], p[:, q0:],
                                    start=(idx == 0), stop=(idx == nk - 1),
                                    skip_group_check=True,
                                )
                                with nc.allow_low_precision(reason="f16 den"):
                                    if idx == 0:
                                        nc.vector.tensor_copy(dacc[:], p[:])
                                    else:
                                        nc.vector.tensor_add(
                                            dacc[:, q0:], dacc[:, q0:], p[:, q0:]
                                        )
                            den = psf.tile([1, 512], F32, name="den",
                                           tag="den", bufs=1)
                            nc.tensor.matmul(
                                den[:], ones[:, 0:1], dacc[:],
                                start=True, stop=True,
                            )
                            dsb = pf.tile([1, 512], F32R, name="dsb",
                                          tag="dsb", bufs=2)
                            with nc.allow_low_precision(reason="fp32r recip"):
                                nc.vector.reciprocal(dsb[:], den[:])
                            rbs = pf.tile([128, 512], F32R, name="rbs",
                                          tag="rbs", bufs=2)
                            if bcast_pe:
                                rb = psf.tile([128, 512], F32, name="rb",
                                              tag="sp", bufs=2)
                                nc.tensor.matmul(
                                    rb[:], ones32[:].bitcast(F32R), dsb[:],
                                    start=True, stop=True,
                                )
                                nc.vector.tensor_copy(rbs[:].bitcast(F32), rb[:])
                            else:
                                nc.gpsimd.partition_broadcast(rbs[:], dsb[:])
                            cs = pf.tile([128, 512], F16, name="cs",
                                         tag="cs", bufs=2)
                            with nc.allow_low_precision(reason="f16 ctx"):
                                nc.vector.tensor_mul(
                                    cs[:], ctx[:], rbs[:].bitcast(F32)
                                )
                            for t in range(512 // RB):
                                jdst = (qb * 512 + t * RB) // RB
                                nc.sync.dma_start(
                                    ctxl[b][jdst, h * 128:(h + 1) * 128, :],
                                    cs[:, t * RB:(t + 1) * RB],
                                )
                        # spread wo prefetch across batch 0 (4 k-tiles per qb)
                        if b == 0:
                            for t in range(4):
                                kt_w = qb * 4 + t
                                nc.sync.dma_start(
                                    wo_sb[:, kt_w, :],
                                    wo.ap()[kt_w * 128:(kt_w + 1) * 128, :],
                                )

                    # per-batch AllToAll, overlapped with next batch compute
                    if "C" in phases:
                        if use_collective:
                            nc.gpsimd.collective_compute(
                                "AllToAll",
                                mybir.AluOpType.bypass,
                                replica_groups=[list(range(N_CORES))],
                                ins=[ctxl[b].opt()],
                                outs=[a2a[b].opt()],
                            )
                        else:  # timing-sim stand-in
                            nc.sync.dma_start(a2a[b][:], ctxl[b][:])
                        # gather: cfull[:, 2j+h, b*RB+m] = a2a[b][j, h*128+p, m]
                        nc.sync.dma_start(
                            cfull_all[:]
                            .rearrange("p (j two) m -> p j two m", two=HPC)
                            [:, :, :, b * RB:(b + 1) * RB],
                            a2a[b][:].rearrange("j (two p) m -> p j two m",
                                                p=128),
                        )

                # -------- Phase C: output projection (both batches) ---------
                if "C" in phases:
                    emit_outproj()

    nc.compile()
    return nc


_NC_CACHE = {}


def _get_nc(key, B, S, with_qkv_bias, with_o_bias, with_kmask, repeat=1):
    if key not in _NC_CACHE:
        _NC_CACHE[key] = build_attention_nc(
            B, S, with_qkv_bias=with_qkv_bias, with_o_bias=with_o_bias,
            with_kmask=with_kmask, repeat=repeat,
        )
    return _NC_CACHE[key]


def _host_masks():
    f = np.arange(128)[None, :]
    p = np.arange(128)[:, None]
    return (f >= p).astype(np.float16)


def prepare_in_maps(hidden_states, sequence_mask, w_qkv, b_qkv, w_o, b_o):
    B, S, D_ = hidden_states.shape
    assert D_ == D
    R = B * S
    NKT = S // 128
    x = np.ascontiguousarray(np.asarray(hidden_states, np.float32).reshape(R, D))
    xt = np.ascontiguousarray(x.T).astype(np.float16)
    w_qkv = np.asarray(w_qkv, np.float32)
    b_qkv = np.asarray(b_qkv, np.float32)
    w_o = np.ascontiguousarray(np.asarray(w_o, np.float32)).astype(np.float16)
    b_o = np.asarray(b_o, np.float32)
    seqm = np.asarray(sequence_mask)

    with_qkv_bias = bool(np.any(b_qkv != 0))
    with_o_bias = bool(np.any(b_o != 0))
    with_kmask = not bool(np.all(seqm))

    masks = _host_masks()
    in_maps = []
    for c in range(N_CORES):
        h0 = HPC * c
        qcols = np.arange(h0 * 128, (h0 + HPC) * 128)
        kcols = qcols + D
        vcols = qcols + 2 * D
        m = {
            "xt": xt,
            "wqk": np.ascontiguousarray(
                w_qkv[:, np.concatenate([qcols, kcols])]).astype(np.float16),
            "wv": np.ascontiguousarray(w_qkv[:, vcols]).astype(np.float16),
            "wo": w_o,
            "masks": masks,
            "eye": np.eye(128, dtype=np.float16),
        }
        if with_qkv_bias:
            bqk = b_qkv[np.concatenate([qcols, kcols])]
            m["bqkT"] = np.ascontiguousarray(bqk.reshape(4, 128).T)
            m["bvrow"] = np.ascontiguousarray(
                b_qkv[vcols].reshape(1, 256)).astype(np.float16)
        if with_o_bias:
            m["boT"] = np.ascontiguousarray(b_o.reshape(D // 128, 128).T)
        if with_kmask:
            km = seqm.astype(np.float32).reshape(B, NKT, 128)
            m["kmaskT"] = np.ascontiguousarray(
                km.transpose(2, 0, 1).reshape(128, B * NKT)
            ).astype(np.float16)
        in_maps.append(m)
    return in_maps, (with_qkv_bias, with_o_bias, with_kmask)


def assemble_output(outTs, B, S):
    """outTs: per-core [D, RC] arrays; returns [B, S, D] f32."""
    R = B * S
    RB = S // N_CORES
    out = np.empty((R, D), np.float32)
    for c, oT in enumerate(outTs):
        blk = np.asarray(oT, np.float32).T  # [RC, D]
        for b in range(B):
            out[b * S + c * RB:(b * S + (c + 1) * RB)] = \
                blk[b * RB:(b + 1) * RB]
    return out.reshape(B, S, D)


def run(hidden_states, sequence_mask, w_qkv, b_qkv, w_o, b_o, **run_kwargs):
    B, S, _ = hidden_states.shape
    in_maps, flags = prepare_in_maps(
        hidden_states, sequence_mask, w_qkv, b_qkv, w_o, b_o
    )
    nc = _get_nc((B, S) + flags, B, S, *flags)
    res = run_bass_kernel_spmd(
        nc, in_maps, core_ids=list(range(N_CORES)), **run_kwargs
    )
    out = assemble_output([r["outT"] for r in res.results], B, S)
    return out, res


def kernel(**inputs):
    out, _ = run(**inputs)
    return out


# revision 51
# speedup vs baseline: 1.0367x; 1.0367x over previous
"""Causal self-attention (B=2, S=2048, D=2048, H=16) on 8 TRN2 NeuronCores.

Sharding: tensor-parallel over heads (2 heads/core) for QKV projection and
attention; per-batch AllToAll redistributes per-head context to per-row
shards (overlapped with the next batch's compute); the output projection is
row-parallel; the host concatenates the 8 row shards.

Data layouts (per core c, heads h0=2c, h1=2c+1):
  xt    [D, R]     X^T, full (R = B*S rows), fp16
  wqk   [D, 512]   w_qkv columns [q_h0 | q_h1 | k_h0 | k_h1] (128 each), fp16
  wv    [D, 256]   w_qkv columns [v_h0 | v_h1], fp16
  wo    [D, D]     full output projection weight, fp16
  masks [128, 128] single causal triangle m[p,f] = (f >= p), fp16

Pipeline (per core, per batch b):
  A) stream xt per q-block (512 rows; startup chunk-interleaved with wqk so
     the first chains stream with the DMA), QKV via PE chains; Q tiles stay
     in SBUF, K/V cached per head (ACT drains PSUM, keeping DVE off the
     critical path).
  B) per (h, q-block): scores k-tiles (diagonal tiles first, q-sliced to the
     causal extent), exp on ACT (no max subtraction: scores ~ N(0,1)),
     triangle mask on the 128-wide diagonal strip only, ctx accumulation in
     PSUM, fp16 denominator accumulation on DVE (2-byte fast mode), fp32r
     reciprocal + PE broadcast matmul (NOT gpsimd partition_broadcast: the
     gpsimd queue hosts the collectives and blocks), normalize+cast on DVE.
  C) after batch b: ONE merged AllToAll per batch ([8, 256, 256]: both heads)
     overlapped with batch b+1 compute, gathered into a combined cfull
     [128, 16, 512]; per-batch free=256 out-projection chains — batch-0's
     run right at phase-F end and cover the last batch's A2A+gather latency
     (no PE bubble), draining into a staged SBUF tile flushed via gpsimd;
     batch-1 drains per-ob in bf16.

Output outT [D, 512] bf16 per core; columns = [b0 rows c*256..(c+1)*256,
b1 rows 2048+c*256..]; host reassembles and casts to f32.

HW timing: build with repeat=N to measure on-device time as the slope
between repeat=1 and repeat=N wall times (see test.py).
"""

import numpy as np

import concourse.bass as bass
import concourse.mybir as mybir
import concourse.tile as tile
from concourse import bacc
from concourse.bass_utils import run_bass_kernel_spmd

F32 = mybir.dt.float32
F32R = mybir.dt.float32r
F16 = mybir.dt.float16
BF = mybir.dt.bfloat16
AF = mybir.ActivationFunctionType

N_CORES = 8
D = 2048
H = 16
DK = 128
HPC = H // N_CORES  # heads per core = 2
SCALE = 1.0 / float(DK) ** 0.5


def build_attention_nc(B, S, with_qkv_bias=False, with_o_bias=False, with_kmask=False,
                       use_collective=True, phases="FC", repeat=1,
                       bcast_pe=True, flush_sync=False, v_transpose=False,
                       oproj_split=True):
    R = B * S
    RC = R // N_CORES          # out rows per core (512)
    RB = S // N_CORES          # rows per core per batch (256)
    KD = D // 128              # contraction tiles (16)
    NQ = S // 512              # q-blocks per batch (4)
    NKT = S // 128             # k-tiles per batch (16)
    assert R % N_CORES == 0 and S % 512 == 0 and RB * B == RC

    nc = bacc.Bacc(
        "TRN2", target_bir_lowering=False, debug=False, num_devices=N_CORES
    )

    xt = nc.dram_tensor("xt", [D, R], F16, kind="ExternalInput")
    wqk = nc.dram_tensor("wqk", [D, 4 * 128], F16, kind="ExternalInput")
    wv = nc.dram_tensor("wv", [D, 2 * 128], F16, kind="ExternalInput")
    wo = nc.dram_tensor("wo", [D, D], F16, kind="ExternalInput")
    masks = nc.dram_tensor("masks", [128, 128], F16, kind="ExternalInput")
    eye = nc.dram_tensor("eye", [128, 128], F16, kind="ExternalInput")
    if with_qkv_bias:
        bqkT = nc.dram_tensor("bqkT", [128, 4], F32, kind="ExternalInput")
        bvrow = nc.dram_tensor("bvrow", [1, 256], F16, kind="ExternalInput")
    if with_o_bias:
        boT = nc.dram_tensor("boT", [128, KD], F32, kind="ExternalInput")
    if with_kmask:
        kmaskT = nc.dram_tensor("kmaskT", [128, B * NKT], F16, kind="ExternalInput")
    outT = nc.dram_tensor("outT", [D, RC], BF, kind="ExternalOutput")

    with tile.TileContext(nc, num_cores=N_CORES) as tc:
        with tc.tile_pool(name="dram", bufs=1, space="DRAM") as dpool, \
             tc.tile_pool(name="pf", bufs=1) as pf, \
             tc.tile_pool(name="psf", bufs=1, space="PSUM") as psf:
            ctxl = [dpool.tile([N_CORES, HPC * 128, RB], F16, name=f"ctxl_b{b}")
                    for b in range(B)]
            a2a = [dpool.tile([N_CORES, HPC * 128, RB], F16, name=f"a2a_b{b}")
                   for b in range(B)]

            for rep in range(repeat):
                ones = pf.tile([128, 128], F16, name="ones", tag="ones", bufs=1)
                nc.vector.memset(ones[:], 1.0)
                if bcast_pe:
                    ones32 = pf.tile([1, 128], F32, name="ones32",
                                     tag="ones32", bufs=1)
                    nc.vector.memset(ones32[:], 1.0)
                masks_sb = pf.tile([128, 128], F16, name="masks_sb",
                                   tag="masks", bufs=1)
                eye_sb = pf.tile([128, 128], F16, name="eye_sb",
                                 tag="eye", bufs=1)
                if v_transpose:
                    nc.sync.dma_start(eye_sb[:], eye.ap())
                if with_qkv_bias:
                    bqk_sb = pf.tile([128, 4], F32, name="bqk_sb", tag="bqk", bufs=1)
                    nc.sync.dma_start(bqk_sb[:], bqkT.ap())
                    bv_sb = pf.tile([1, 256], F16, name="bv_sb", tag="bv", bufs=1)
                    nc.sync.dma_start(bv_sb[:], bvrow.ap())
                if with_o_bias:
                    bo_sb = pf.tile([128, KD], F32, name="bo_sb", tag="bo", bufs=1)
                    nc.sync.dma_start(bo_sb[:], boT.ap())
                if with_kmask:
                    km_sb = pf.tile([128, B * NKT], F16, name="km_sb",
                                    tag="km", bufs=1)
                    nc.sync.dma_start(km_sb[:], kmaskT.ap())

                wqk_sb = pf.tile([128, KD, 512], F16, name="wqk_sb",
                                 tag="wqk", bufs=1)
                wqk_r = wqk.ap().rearrange("(t p) m -> p t m", p=128)
                wv_sb = pf.tile([128, KD, 256], F16, name="wv_sb",
                                tag="wv", bufs=1)
                wo_sb = pf.tile([128, KD, D], F16, name="wo_sb", tag="wo", bufs=1)
                xt_r = xt.ap().rearrange("(t p) m -> p t m", p=128)
                kcache = [
                    pf.tile([128, S], F16, name=f"kcache{h}", tag=f"kc{h}", bufs=1)
                    for h in range(HPC)
                ]
                vcache = [
                    pf.tile([128, NKT, 128], F16, name=f"vcache{h}",
                            tag=f"vc{h}", bufs=1)
                    for h in range(HPC)
                ]

                # ---------------- Phase F: QKV + attention ----------------
                cfull_all = pf.tile([128, KD, RC], F16, name="cfull",
                                    tag="cfull", bufs=1)

                def emit_outproj():
                    for ob in range(KD):
                        pso = psf.tile([128, RC], F32, name="pso",
                                       tag="mm", bufs=2)
                        for kt in range(KD):
                            nc.tensor.matmul(
                                pso[:],
                                wo_sb[:, kt, ob * 128:(ob + 1) * 128],
                                cfull_all[:, kt, :],
                                start=(kt == 0),
                                stop=(kt == KD - 1),
                            )
                        evo = pf.tile([128, RC], BF, name="evo",
                                      tag="evo", bufs=4)
                        with nc.allow_low_precision(reason="bf16 out"):
                            if with_o_bias:
                                nc.vector.tensor_scalar_add(
                                    evo[:], pso[:], bo_sb[:, ob:ob + 1]
                                )
                            else:
                                nc.scalar.copy(evo[:], pso[:])
                        nc.sync.dma_start(
                            outT.ap()[ob * 128:(ob + 1) * 128, :],
                            evo[:],
                        )

                def emit_outproj_split():
                    # per-batch free=256 chains: batch-0 chains run right at
                    # phase-F end and cover the last batch's A2A+gather
                    # latency (no PE bubble); batch-0 drains into a staged
                    # SBUF tile (no buffer-reuse guards on the in-order DMA
                    # queues), flushed via gpsimd after batch-1's gather.
                    osb = pf.tile([128, KD, RB], BF, name="osb",
                                  tag="osb", bufs=1)
                    for bb in range(B):
                        staged = bb < B - 1
                        if bb == B - 1 and B > 1:
                            nc.gpsimd.dma_start(
                                outT.ap()
                                .rearrange("(t p) m -> p t m", p=128)
                                [:, :, 0:RB],
                                osb[:],
                            )
                        for ob in range(KD):
                            pso = psf.tile([128, RB], F32, name="pso",
                                           tag="mm", bufs=2)
                            for kt in range(KD):
                                nc.tensor.matmul(
                                    pso[:],
                                    wo_sb[:, kt, ob * 128:(ob + 1) * 128],
                                    cfull_all[:, kt, bb * RB:(bb + 1) * RB],
                                    start=(kt == 0),
                                    stop=(kt == KD - 1),
                                )
                            dst = (osb[:, ob, :] if staged else
                                   pf.tile([128, RB], BF, name="evo",
                                           tag="evo", bufs=6)[:])
                            with nc.allow_low_precision(reason="bf16 out"):
                                if with_o_bias:
                                    nc.vector.tensor_scalar_add(
                                        dst, pso[:], bo_sb[:, ob:ob + 1]
                                    )
                                else:
                                    nc.scalar.copy(dst, pso[:])
                            if not staged:
                                nc.sync.dma_start(
                                    outT.ap()[ob * 128:(ob + 1) * 128,
                                              bb * RB:(bb + 1) * RB],
                                    dst,
                                )

                if "F" in phases:
                 for b in range(B):
                    for qb in range(NQ):
                        xt_q = pf.tile([128, KD, 512], F16, name="xt_q",
                                       tag="xt", bufs=3)
                        xcols = slice(b * S + qb * 512, b * S + (qb + 1) * 512)
                        if b == 0 and qb == 0:
                            # startup: chunked so the first Q/K chain pair
                            # streams with the DMA; chunk size balances the
                            # per-DMA descriptor cost vs transfer latency
                            nc.sync.dma_start(
                                wqk_sb[:, 0:4, 0:256], wqk_r[:, 0:4, 0:256]
                            )
                            nc.sync.dma_start(
                                xt_q[:, 0:4, :], xt_r[:, 0:4, xcols]
                            )
                            nc.sync.dma_start(
                                xt_q[:, 4:8, :], xt_r[:, 4:8, xcols]
                            )
                            nc.sync.dma_start(
                                wqk_sb[:, 4:16, 0:256], wqk_r[:, 4:16, 0:256]
                            )
                            nc.sync.dma_start(
                                xt_q[:, 8:12, :], xt_r[:, 8:12, xcols]
                            )
                            nc.sync.dma_start(
                                wqk_sb[:, :, 256:512], wqk_r[:, :, 256:512]
                            )
                            nc.sync.dma_start(
                                xt_q[:, 12:16, :], xt_r[:, 12:16, xcols]
                            )
                            nc.sync.dma_start(masks_sb[:], masks.ap())
                            # wv only needed once Q/K chains are in flight
                            nc.sync.dma_start(
                                wv_sb[:],
                                wv.ap().rearrange("(t p) m -> p t m", p=128),
                            )
                        else:
                            for g in range(4):
                                nc.sync.dma_start(
                                    xt_q[:, g * 4:(g + 1) * 4, :],
                                    xt_r[:, g * 4:(g + 1) * 4, xcols],
                                )
                        # Q/K chains: m in (q_h0, q_h1, k_h0, k_h1), emitted
                        # k-major over m-pairs so PE streams with arriving xt
                        qtile = []
                        for m0 in (0, 2):
                            pss = [psf.tile([128, 512], F32, name="ps",
                                            tag="mm", bufs=2) for _ in range(2)]
                            for k in range(KD):
                                for mi in range(2):
                                    nc.tensor.matmul(
                                        pss[mi][:],
                                        wqk_sb[:, k, (m0 + mi) * 128:
                                               (m0 + mi + 1) * 128],
                                        xt_q[:, k, :],
                                        start=(k == 0),
                                        stop=(k == KD - 1),
                                    )
                            for mi in range(2):
                                m = m0 + mi
                                if m < 2:
                                    qt = pf.tile([128, 512], F16, name="qt",
                                                 tag="qt", bufs=4)
                                    qtile.append(qt)
                                    dst = qt[:]
                                else:
                                    dst = kcache[m - 2][:, qb * 512:
                                                        (qb + 1) * 512]
                                with nc.allow_low_precision(reason="f16 cache"):
                                    if with_qkv_bias:
                                        nc.vector.tensor_scalar_add(
                                            dst, pss[mi][:], bqk_sb[:, m:m + 1]
                                        )
                                    else:
                                        nc.scalar.copy(dst, pss[mi][:])
                        # V chains
                        if v_transpose and not with_qkv_bias:
                            # feat-major chains (free=512, engine-bound on the
                            # PE sequencer) + PE transpose into row-major cache
                            for h in range(HPC):
                                psvT = psf.tile([128, 512], F32, name="psvT",
                                                tag="mm", bufs=2)
                                for k in range(KD):
                                    nc.tensor.matmul(
                                        psvT[:],
                                        wv_sb[:, k, h * 128:(h + 1) * 128],
                                        xt_q[:, k, :],
                                        start=(k == 0),
                                        stop=(k == KD - 1),
                                    )
                                vT = pf.tile([128, 512], F16, name="vT",
                                             tag="vT", bufs=2)
                                with nc.allow_low_precision(reason="f16 cache"):
                                    nc.scalar.copy(vT[:], psvT[:])
                                for s4 in range(4):
                                    ps_t = psf.tile([128, 128], F16, name="ps_t",
                                                    tag="mmv", bufs=1)
                                    nc.tensor.transpose(
                                        ps_t[:],
                                        vT[:, s4 * 128:(s4 + 1) * 128],
                                        eye_sb[:],
                                    )
                                    nc.scalar.copy(
                                        vcache[h][:, qb * 4 + s4, :], ps_t[:]
                                    )
                        else:
                            for s4 in range(4):
                                psv = psf.tile([128, 256], F32, name="psv",
                                               tag="mmv", bufs=1)
                                for k in range(KD):
                                    nc.tensor.matmul(
                                        psv[:],
                                        xt_q[:, k, s4 * 128:(s4 + 1) * 128],
                                        wv_sb[:, k, :],
                                        start=(k == 0),
                                        stop=(k == KD - 1) and not with_qkv_bias,
                                        skip_group_check=with_qkv_bias,
                                    )
                                if with_qkv_bias:
                                    nc.tensor.matmul(
                                        psv[:], ones[0:1, :], bv_sb[:],
                                        start=False, stop=True,
                                        skip_group_check=True,
                                    )
                                kt_ = qb * 4 + s4
                                for h in range(HPC):
                                    with nc.allow_low_precision(reason="f16 cache"):
                                        nc.scalar.copy(
                                            vcache[h][:, kt_, :],
                                            psv[:, h * 128:(h + 1) * 128],
                                        )
                        # attention for q-block qb, both heads
                        nk = (qb + 1) * 4
                        korder = (
                            list(range(qb * 4, nk)) + list(range(0, qb * 4))
                        )
                        for h in range(HPC):
                            ctx = psf.tile([128, 512], F32, name="ctx",
                                           tag="ctx", bufs=2)
                            dacc = pf.tile([128, 512], F16, name="dacc",
                                           tag="dacc", bufs=2)
                            for idx, kt in enumerate(korder):
                                j = kt - qb * 4  # >=0 on the causal diagonal
                                q0 = j * 128 if j >= 0 else 0
                                sp = psf.tile([128, 512], F32, name="sp",
                                              tag="sp", bufs=2)
                                nc.tensor.matmul(
                                    sp[:, q0:],
                                    kcache[h][:, kt * 128:(kt + 1) * 128],
                                    qtile[h][:, q0:],
                                    start=True,
                                    stop=True,
                                )
                                p = pf.tile([128, 512], F16, name="p",
                                            tag="p", bufs=6)
                                nc.scalar.activation(
                                    p[:, q0:], sp[:, q0:], AF.Exp, scale=SCALE
                                )
                                if j >= 0:
                                    nc.vector.tensor_mul(
                                        p[:, q0:q0 + 128], p[:, q0:q0 + 128],
                                        masks_sb[:],
                                    )
                                if with_kmask:
                                    nc.vector.tensor_scalar_mul(
                                        p[:, q0:], p[:, q0:],
                                        km_sb[:, b * NKT + kt: b * NKT + kt + 1],
                                    )
                                nc.tensor.matmul(
                                    ctx[:, q0:], vcache[h][:, kt, :], p[:, q0:],
                                    start=(idx == 0), stop=(idx == nk - 1),
                                    skip_group_check=True,
                                )
                                with nc.allow_low_precision(reason="f16 den"):
                                    if idx == 0:
                                        nc.vector.tensor_copy(dacc[:], p[:])
                                    else:
                                        nc.vector.tensor_add(
                                            dacc[:, q0:], dacc[:, q0:], p[:, q0:]
                                        )
                            den = psf.tile([1, 512], F32, name="den",
                                           tag="den", bufs=1)
                            nc.tensor.matmul(
                                den[:], ones[:, 0:1], dacc[:],
                                start=True, stop=True,
                            )
                            dsb = pf.tile([1, 512], F32R, name="dsb",
                                          tag="dsb", bufs=2)
                            with nc.allow_low_precision(reason="fp32r recip"):
                                nc.vector.reciprocal(dsb[:], den[:])
                            rbs = pf.tile([128, 512], F32R, name="rbs",
                                          tag="rbs", bufs=2)
                            if bcast_pe:
                                rb = psf.tile([128, 512], F32, name="rb",
                                              tag="sp", bufs=2)
                                nc.tensor.matmul(
                                    rb[:], ones32[:].bitcast(F32R), dsb[:],
                                    start=True, stop=True,
                                )
                                nc.vector.tensor_copy(rbs[:].bitcast(F32), rb[:])
                            else:
                                nc.gpsimd.partition_broadcast(rbs[:], dsb[:])
                            cs = pf.tile([128, 512], F16, name="cs",
                                         tag="cs", bufs=2)
                            with nc.allow_low_precision(reason="f16 ctx"):
                                nc.vector.tensor_mul(
                                    cs[:], ctx[:], rbs[:].bitcast(F32)
                                )
                            for t in range(512 // RB):
                                jdst = (qb * 512 + t * RB) // RB
                                nc.sync.dma_start(
                                    ctxl[b][jdst, h * 128:(h + 1) * 128, :],
                                    cs[:, t * RB:(t + 1) * RB],
                                )
                        # spread wo prefetch across batch 0 (4 k-tiles per qb)
                        if b == 0:
                            for t in range(4):
                                kt_w = qb * 4 + t
                                nc.sync.dma_start(
                                    wo_sb[:, kt_w, :],
                                    wo.ap()[kt_w * 128:(kt_w + 1) * 128, :],
                                )

                    # per-batch AllToAll, overlapped with next batch compute
                    if "C" in phases:
                        if use_collective:
                            nc.gpsimd.collective_compute(
                                "AllToAll",
                                mybir.AluOpType.bypass,
                                replica_groups=[list(range(N_CORES))],
                                ins=[ctxl[b].opt()],
                                outs=[a2a[b].opt()],
                            )
                        else:  # timing-sim stand-in
                            nc.sync.dma_start(a2a[b][:], ctxl[b][:])
                        # gather: cfull[:, 2j+h, b*RB+m] = a2a[b][j, h*128+p, m]
                        nc.sync.dma_start(
                            cfull_all[:]
                            .rearrange("p (j two) m -> p j two m", two=HPC)
                            [:, :, :, b * RB:(b + 1) * RB],
                            a2a[b][:].rearrange("j (two p) m -> p j two m",
                                                p=128),
                        )

                # -------- Phase C: output projection (both batches) ---------
                if "C" in phases:
                    if oproj_split:
                        emit_outproj_split()
                    else:
                        emit_outproj()

    nc.compile()
    return nc


_NC_CACHE = {}


def _get_nc(key, B, S, with_qkv_bias, with_o_bias, with_kmask, repeat=1):
    if key not in _NC_CACHE:
        _NC_CACHE[key] = build_attention_nc(
            B, S, with_qkv_bias=with_qkv_bias, with_o_bias=with_o_bias,
            with_kmask=with_kmask, repeat=repeat,
        )
    return _NC_CACHE[key]


def _host_masks():
    f = np.arange(128)[None, :]
    p = np.arange(128)[:, None]
    return (f >= p).astype(np.float16)


def prepare_in_maps(hidden_states, sequence_mask, w_qkv, b_qkv, w_o, b_o):
    B, S, D_ = hidden_states.shape
    assert D_ == D
    R = B * S
    NKT = S // 128
    x = np.ascontiguousarray(np.asarray(hidden_states, np.float32).reshape(R, D))
    xt = np.ascontiguousarray(x.T).astype(np.float16)
    w_qkv = np.asarray(w_qkv, np.float32)
    b_qkv = np.asarray(b_qkv, np.float32)
    w_o = np.ascontiguousarray(np.asarray(w_o, np.float32)).astype(np.float16)
    b_o = np.asarray(b_o, np.float32)
    seqm = np.asarray(sequence_mask)

    with_qkv_bias = bool(np.any(b_qkv != 0))
    with_o_bias = bool(np.any(b_o != 0))
    with_kmask = not bool(np.all(seqm))

    masks = _host_masks()
    in_maps = []
    for c in range(N_CORES):
        h0 = HPC * c
        qcols = np.arange(h0 * 128, (h0 + HPC) * 128)
        kcols = qcols + D
        vcols = qcols + 2 * D
        m = {
            "xt": xt,
            "wqk": np.ascontiguousarray(
                w_qkv[:, np.concatenate([qcols, kcols])]).astype(np.float16),
            "wv": np.ascontiguousarray(w_qkv[:, vcols]).astype(np.float16),
            "wo": w_o,
            "masks": masks,
            "eye": np.eye(128, dtype=np.float16),
        }
        if with_qkv_bias:
            bqk = b_qkv[np.concatenate([qcols, kcols])]
            m["bqkT"] = np.ascontiguousarray(bqk.reshape(4, 128).T)
            m["bvrow"] = np.ascontiguousarray(
                b_qkv[vcols].reshape(1, 256)).astype(np.float16)
        if with_o_bias:
            m["boT"] = np.ascontiguousarray(b_o.reshape(D // 128, 128).T)
        if with_kmask:
            km = seqm.astype(np.float32).reshape(B, NKT, 128)
            m["kmaskT"] = np.ascontiguousarray(
                km.transpose(2, 0, 1).reshape(128, B * NKT)
            ).astype(np.float16)
        in_maps.append(m)
    return in_maps, (with_qkv_bias, with_o_bias, with_kmask)


def assemble_output(outTs, B, S):
    """outTs: per-core [D, RC] arrays; returns [B, S, D] f32."""
    R = B * S
    RB = S // N_CORES
    out = np.empty((R, D), np.float32)
    for c, oT in enumerate(outTs):
        blk = np.asarray(oT, np.float32).T  # [RC, D]
        for b in range(B):
            out[b * S + c * RB:(b * S + (c + 1) * RB)] = \
                blk[b * RB:(b + 1) * RB]
    return out.reshape(B, S, D)


def run(hidden_states, sequence_mask, w_qkv, b_qkv, w_o, b_o, **run_kwargs):
    B, S, _ = hidden_states.shape
    in_maps, flags = prepare_in_maps(
        hidden_states, sequence_mask, w_qkv, b_qkv, w_o, b_o
    )
    nc = _get_nc((B, S) + flags, B, S, *flags)
    res = run_bass_kernel_spmd(
        nc, in_maps, core_ids=list(range(N_CORES)), **run_kwargs
    )
    out = assemble_output([r["outT"] for r in res.results], B, S)
    return out, res


def kernel(**inputs):
    out, _ = run(**inputs)
    return out
